# revision 1
# baseline (speedup 1.0000x reference)
r"""Circulant layer kernel for Trainium2 (8 NeuronCores).

Math: reference computes mv1 + mv2 where
  mv1 = batch_circulant(b) @ d,  mv2 = batch_circulant(d) @ b,
with d = des @ K, b = body @ K.  Both are the circular convolution of d and b
(circular convolution is commutative), so  out = 2 * circconv(d, b).

circconv via DFT:  out = 2 * Re(IDFT(DFT(d) * DFT(b))).
DFT/IDFT are realized as dense matmuls with host-generated constant
cos/sin matrices (input-independent constants).

Sharding: each of the 8 cores owns 128 of the 1024 DFT frequencies.
Per core c:
  KC_c   = K @ CC_c            (1024k x 256s)   fused projection+forward DFT
  DT_c   = KC_c^T @ des^T      (256s x 128b)    \  shares stationary weights
  BT_c   = KC_c^T @ body^T     (256s x 128b)    /
  PT_c   = complex-mult(DT_c, BT_c)             (256s x 128b)  on VectorE
  part_c = (PT_c^T @ G_c)                       (128b x 1024)  inverse DFT
Host sums the 8 partials (unshard).
"""

import numpy as np

import concourse.bass as bass
import concourse.mybir as mybir
import concourse.tile as tile
from concourse.bass_utils import run_bass_kernel_spmd
from concourse.tile_rust import add_dep_helper

B = 128        # batch
D_IN = 1024    # input feature dim (contraction k)
N = 1024       # output feature dim (conv length j) == #frequencies
N_CORES = 8
FPC = N // N_CORES  # frequencies per core (complex)
S = 2 * FPC         # freq slots per core: [0:FPC]=real(cos), [FPC:2FPC]=imag(-sin)

F32 = mybir.dt.float32
F32R = mybir.dt.float32r
BF16 = mybir.dt.bfloat16

# Matmul operand precision: "bf16" (fastest; ~5e-3 rel err), "f32r"
# (single-pass TF32-like; ~3e-4), "f32" (two-pass full fp32; ~7e-7).
import os as _os
MM_PREC = _os.environ.get("CIRC_MM_PREC", "f32r")
MM_DT = {"bf16": BF16, "f32r": F32R, "f32": F32}[MM_PREC]


def _np_in(a):
    """Cast to the matmul precision; bf16 data is shipped packed in fp32
    words (DMA is element-rate-bound: 2-byte elements run at half rate)."""
    import ml_dtypes
    a = np.ascontiguousarray(np.asarray(a, dtype=np.float32))
    if MM_PREC != "bf16":
        return a
    bf = np.ascontiguousarray(a.astype(ml_dtypes.bfloat16))
    return bf.view(np.uint8).reshape(a.shape[0], -1).view(np.float32)

# Number of fp32 transport words per logical input element.
PACK = 2 if MM_PREC == "bf16" else 1
# Transport dtype: bf16 ships packed in fp32 words; f32/f32r ship natively
# (the fp32r verifier requires the producing DMA to be f32r-typed).
TR_DT = F32 if MM_PREC == "bf16" else MM_DT

# Stashed by kernel() for test harnesses that want profiling info.
LAST_RESULT = None

_nc_cache = {}


def _build_nc():
    """Build the (single-program) Bass module run on all 8 cores."""
    nc = bass.Bass(target_bir_lowering=True)

    # Packed inputs: tensors consumed together share one DMA (keeps the
    # per-matmul semaphore-wait count within the ISA limit).
    #   ktcc[j, :D_IN] = K^T,  ktcc[j, D_IN:] = CC   (both indexed by j)
    #   dbt[k, :B] = des^T,    dbt[k, B:] = body^T   (both indexed by k)
    # All inputs are host-packed per SBUF partition: row p holds everything
    # partition p receives, contiguously, so each DMA moves 128 long
    # contiguous rows (DMA throughput is descriptor-rate-bound otherwise).
    XW = (D_IN + S) // PACK
    DW = 2 * B // PACK
    GW = N // PACK
    # ktcc in two contiguous halves, one per HWDGE channel (SP / ACT).
    # Channels are FIFO with ~4-5us latency PER TRANSFER, so one big
    # transfer per channel beats several small ones.
    ktcc_q = [nc.declare_dram_parameter(f"ktcc{i}", [128, 4 * XW], TR_DT, False)
              for i in range(2)]
    # aux = [dbt rows | g rows] packed per partition; goes via gpsimd SWDGE.
    aux = nc.declare_dram_parameter("aux", [128, 8 * DW + 2 * GW], TR_DT, False)  # (s, t) inv DFT rows
    out = nc.declare_dram_parameter("out", [B, N], F32, isOutput=True)
    warm_scratch = nc.dram_tensor("warm_scratch", [1, 4], F32)

    JC = N // 128      # 8 chunks over j (contraction of KC stage)
    KB = D_IN // 128   # 8 blocks over k (output partitions of KC stage)
    SB = S // 128      # 2 blocks over freq slots

    with tile.TileContext(nc) as tc:
        with (
            tc.tile_pool(name="main", bufs=1) as pool,
            tc.tile_pool(name="psum", bufs=1, space="PSUM") as pp,
        ):
            # ---- inputs -> SBUF ----
            # At most 7 input DMAs so the output store lands on the 8th,
            # otherwise-unused HW queue: a DMACopy can carry only ONE sync
            # wait, and the store needs its data-dependency wait — it must
            # not also need a queue-slot wait.
            in_dmas = []
            # All input transfers ride ONE serial SP chain: parallel channels
            # all pay the full ~12us proxy latency, while a serial chain
            # pipelines (first chunk lands ~6us in, rest follow every ~3us).
            ktcc_sb = [pool.tile([128, 4, XW], TR_DT, tag=f"ktcc{q}", name=f"ktcc{q}")
                       for q in range(2)]
            for q in range(2):
                in_dmas.append(nc.sync.dma_start(ktcc_sb[q][:], ktcc_q[q][:, :]))
            aux_raw = pool.tile([128, 8 * DW + 2 * GW], TR_DT, tag="auxr", name="auxr")
            in_dmas.append(nc.sync.dma_start(aux_raw[:], aux[:, :]))
            ktcc_v = [t.bitcast(MM_DT) for t in ktcc_sb]
            kt_sb = [ktcc_v[j // 4][:, j % 4, :D_IN] for j in range(JC)]
            cc_sb = [ktcc_v[j // 4][:, j % 4, D_IN:] for j in range(JC)]
            dbt_sb = pool.tile([128, KB, DW], TR_DT, tag="dbt", name="dbt")
            nc.vector.tensor_copy(dbt_sb[:], aux_raw[:, :8 * DW].rearrange("p (kb w) -> p kb w", kb=KB))
            dbt_v = dbt_sb.bitcast(MM_DT)
            g_stage = pool.tile([128, SB, GW], TR_DT, tag="gst", name="gst")
            nc.vector.tensor_copy(g_stage[:], aux_raw[:, 8 * DW:].rearrange("p (sb w) -> p sb w", sb=SB))
            g_sb = [g_stage.bitcast(MM_DT)[:, s, :] for s in range(SB)]

            # ---- PE warmup: keep the HAM clock un-throttled while DMAs
            # stream in, so the real matmuls all run at 2.4 GHz. Dead-code
            # proofed by a tiny gpsimd DMA of the result to scratch DRAM.
            wz = pool.tile([128, 640], BF16, tag="wz", name="wz")
            nc.gpsimd.memset(wz[:], 0.0)
            wps = pp.tile([128, 512], F32, tag="wps", name="wps")
            for w in range(20):
                nc.tensor.matmul(wps[:], wz[:, :128], wz[:, 128:640],
                                 start=True, stop=True)
            wsb = pool.tile([128, 4], F32, tag="wsb", name="wsb")
            nc.vector.tensor_copy(wsb[:], wps[:, :4])
            warm_dma = nc.gpsimd.dma_start(warm_scratch[:, :], wsb[:1, :])

            # ---- stage 1 + stage 2 interleaved ----
            # KC[k, s] = sum_j KT[j, k] * CC[j, s]; as soon as kc chunk kb is
            # cast to bf16, both stage-2 accumulations consume it, hiding the
            # stage-2 matmuls inside stage-1's DMA-paced gaps.
            kc_sb = [pool.tile([128, S], MM_DT, tag=f"kc{kb}", name=f"kc{kb}") for kb in range(KB)]
            db_all = pool.tile([128, SB, 2 * B], F32, tag="dball", name="dball")
            db_ps = [pp.tile([128, 2 * B], F32, tag=f"dbp{sb}", name=f"dbp{sb}")
                     for sb in range(SB)]
            for kb in range(KB):
                ps = pp.tile([128, S], F32, tag="kcp", name=f"kcp{kb}", bufs=2)
                for j in range(JC):
                    nc.tensor.matmul(
                        ps[:],
                        kt_sb[j][:, kb * 128:(kb + 1) * 128],
                        cc_sb[j][:],
                        start=(j == 0),
                        stop=(j == JC - 1),
                    )
                nc.vector.tensor_copy(kc_sb[kb][:], ps[:])
                if MM_PREC == "bf16":
                    # interleave stage-2 into stage-1's DMA-paced gaps; for
                    # f32/f32r the serialized 4-byte weight loads make this
                    # interleave a net loss, so run stage 2 afterwards.
                    for sb in range(SB):
                        nc.tensor.matmul(db_ps[sb][:],
                                         kc_sb[kb][:, sb * 128:(sb + 1) * 128],
                                         dbt_v[:, kb, :],
                                         start=(kb == 0), stop=(kb == KB - 1))
            if MM_PREC != "bf16":
                for sb in range(SB):
                    for kb in range(KB):
                        nc.tensor.matmul(db_ps[sb][:],
                                         kc_sb[kb][:, sb * 128:(sb + 1) * 128],
                                         dbt_v[:, kb, :],
                                         start=(kb == 0), stop=(kb == KB - 1))
            for sb in range(SB):
                nc.vector.tensor_copy(db_all[:, sb, :], db_ps[sb][:])

            # ---- stage 3: complex pointwise multiply (on freq partitions) ----
            # t01 = [Dr*Br, Dr*Bi], t23 = [Di*Bi, Di*Br]
            # Pr = t01[0] - t23[0],  Pi = t01[1] + t23[1]
            t01 = pool.tile([128, 2, B], F32, tag="t01", name="t01")
            t23 = pool.tile([128, 2, B], F32, tag="t23", name="t23")
            pt = pool.tile([128, 2, B], MM_DT, tag="pt", name="pt")
            dr_b = db_all[:, 0, :B][:, None, :].to_broadcast((128, 2, B))
            di_b = db_all[:, 1, :B][:, None, :].to_broadcast((128, 2, B))
            nc.vector.tensor_mul(t01[:], dr_b, db_all[:, :, B:])
            nc.vector.tensor_mul(t23[:], di_b, db_all[:, ::-1, B:])
            nc.vector.tensor_sub(pt[:, 0, :], t01[:, 0, :], t23[:, 0, :])
            nc.vector.tensor_add(pt[:, 1, :], t01[:, 1, :], t23[:, 1, :])
            pt_sb = [pt[:, sb, :] for sb in range(SB)]

            # ---- stage 4: part = PT^T @ G ----
            out_sb = pool.tile([128, N], F32, tag="outsb", name="outsb")
            last_mm = last_cp = None
            for h in range(2):
                o_ps = pp.tile([128, 512], F32, tag="op", name=f"op{h}", bufs=2)
                for sb in range(SB):
                    last_mm = nc.tensor.matmul(
                        o_ps[:],
                        pt_sb[sb],
                        g_sb[sb][:, h * 512:(h + 1) * 512],
                        start=(sb == 0),
                        stop=(sb == SB - 1),
                    )
                last_cp = nc.vector.tensor_copy(out_sb[:, h * 512:(h + 1) * 512], o_ps[:])
            store_a = nc.sync.dma_start(out[:, :512], out_sb[:, :512])
            store_b = nc.scalar.dma_start(out[:, 512:], out_sb[:, 512:])

            # TileContext's exit emits one tail Drain waiting on every
            # outstanding semaphore; walrus caps instructions at ONE sync
            # wait.  Pre-absorb every tick into SP's clock with a chain of
            # single-wait drains so the tail drain needs none.
            prev = None
            for dep in [*in_dmas, warm_dma, store_a, store_b, last_mm, last_cp]:
                dr = nc.sync.drain(fusable=False)
                add_dep_helper(dr.ins, dep.ins, sync=True,
                               reason="tail: absorb tick into SP clock")
                if prev is not None:
                    add_dep_helper(dr.ins, prev.ins, sync=False,
                                   reason="tail: keep drain chain ordered")
                prev = dr

    return nc


def _dft_constants():
    """Per-core forward (CC) and inverse (G) DFT matrices, float32."""
    j = np.arange(N, dtype=np.float64)
    ccs, gs = [], []
    for c in range(N_CORES):
        f = np.arange(c * FPC, (c + 1) * FPC, dtype=np.float64)
        ang = 2.0 * np.pi * np.outer(j, f) / N          # (j, f)
        cc = np.concatenate([np.cos(ang), -np.sin(ang)], axis=1)   # (N, S)
        # inverse: out[k] = (2/N) * sum_f [Pr cos(2pi f k/N) - Pi sin(2pi f k/N)]
        angT = ang.T                                     # (f, k)
        gr = (2.0 / N) * np.cos(angT)
        gi = -(2.0 / N) * np.sin(angT)
        gmat = np.concatenate([gr, gi], axis=0)          # (S, N)
        ccs.append(np.ascontiguousarray(cc, dtype=np.float32))
        gs.append(np.ascontiguousarray(gmat, dtype=np.float32))
    return ccs, gs


def _partition_pack(a):
    """(R, W) with R = n*128 -> (128, n*W): row p = concat of chunk rows p."""
    r, w = a.shape
    n = r // 128
    return np.ascontiguousarray(
        a.reshape(n, 128, w).transpose(1, 0, 2).reshape(128, n * w))


def kernel(des, body, kernel):
    global LAST_RESULT
    K = np.asarray(kernel, dtype=np.float32)
    kt_np = K.T  # (j, k)
    dbt_np = _partition_pack(_np_in(np.concatenate(
        [np.asarray(des, dtype=np.float32).T, np.asarray(body, dtype=np.float32).T],
        axis=1,
    )))  # (k, 2B) packed
    ccs, gs = _dft_constants()
    ktccs = [
        _partition_pack(_np_in(np.concatenate([kt_np, ccs[c]], axis=1)))
        for c in range(N_CORES)
    ]
    half = ktccs[0].shape[1] // 2
    auxs = [
        np.ascontiguousarray(
            np.concatenate([dbt_np, _partition_pack(_np_in(gs[c]))], axis=1))
        for c in range(N_CORES)
    ]

    if "nc" not in _nc_cache:
        _nc_cache["nc"] = _build_nc()
    nc = _nc_cache["nc"]

    in_maps = [
        {**{f"ktcc{i}": np.ascontiguousarray(ktccs[c][:, i * half:(i + 1) * half])
            for i in range(2)},
         "aux": auxs[c]}
        for c in range(N_CORES)
    ]
    res = run_bass_kernel_spmd(nc, in_maps, list(range(N_CORES)))
    LAST_RESULT = res
    out = np.zeros((B, N), dtype=np.float32)
    for r in res.results:
        out += r["out"]
    return out



# revision 16
# speedup vs baseline: 1.0571x; 1.0571x over previous
r"""Circulant layer kernel for Trainium2 (8 NeuronCores) — v2.

Math: reference computes mv1 + mv2 = 2 * circconv(d, b) with
d = des @ K, b = body @ K.  Realized via a real-input (half-spectrum)
DFT: only frequencies f = 0..512 are computed (conjugate symmetry),
weights folded into the inverse matrix.

Sharding: 513 frequencies over 8 cores.  Cores 0..7 own f = 64c..64c+63;
the Nyquist f=512 rides in core 0's slot-0 imaginary column (sin(0)=0 is
dead), with a generalized 3-product inverse (G3) that keeps the SPMD
program uniform:
  m0 = Dr*Br, m1 = Di*Bi, mC = Dr*Bi + Di*Br  (per slot t)
  out += m0 @ A + m1 @ B + mC @ C             (A/B/C rows are host consts)
For a normal slot (freq f, w=4/N): A = w cos, B = -w cos, C = -w sin.
For core-0 slot 0: m0 = D0*B0 (A = 2/N), m1 = D512*B512 (B = 2/N*(-1)^j),
C = 0.

Per-core pipeline (all matmul operands bf16, PSUM f32):
  S1  KC^T[s,k]   = sum_j CC[j,s]^T-stationary x KT[j,k]-moving (8 mm, ap 1024)
  T1  KC chunks   = PE-transpose of KC^T                        (8 transposes)
  S2  DB[s,2B]    = sum_k KC[k,s]-stationary x [desT|bodyT]     (8 mm, ap 256)
  T2  dT,bT[b,s]  = PE-transpose of DB halves
  PW  ptA=[m0|m1], ptC=mC  on VectorE
  T3  PA,PC       = PE-transpose of ptA, ptC
  S4  out[b,j]    = PA^T @ G3a + PC^T @ G3b                     (4 mm, ap 512)
Store is bf16 packed in f32 words; host sums the 8 partials.
"""

import numpy as np
import ml_dtypes

import concourse.bass as bass
import concourse.mybir as mybir
import concourse.tile as tile
from concourse.bass_utils import run_bass_kernel_spmd
from concourse.tile_rust import add_dep_helper

B = 128        # batch
D_IN = 1024    # input feature dim (contraction k)
N = 1024       # output feature dim (conv length j)
N_CORES = 8
FPC = 64       # complex frequency slots per core
S = 2 * FPC    # 128 freq columns per core: [0:64]=re(cos), [64:128]=im(-sin)

F32 = mybir.dt.float32
BF16 = mybir.dt.bfloat16

KC_CH = 8      # j-chunks in stage 1 / k-chunks in stage 2
N_KT_DMA = 4   # kt split into this many DMAs (2 j-chunks each)

LAST_RESULT = None
_nc_cache = {}


def _build_nc(warm_iters=20):
    nc = bass.Bass(target_bir_lowering=True)

    # All wire tensors are bf16 packed two-per-f32-word (DMA is element-rate
    # bound).  Per-partition packing: row p holds everything partition p gets.
    # Every instruction may carry ONE semaphore wait, so tensors consumed
    # together share a DMA (cc rides with kt) or are re-staged through DVE.
    CW = N + S                    # bf16 words per (kt chunk | cc chunk) pair
    hdr = nc.declare_dram_parameter("hdr", [128, 128 // 2], F32, False)
    kts = [nc.declare_dram_parameter(f"kt{q}", [128, 2 * CW // 2], F32, False)
           for q in range(N_KT_DMA)]
    dbg = nc.declare_dram_parameter("dbg", [128, (KC_CH * 2 * B + N) // 2], F32, False)
    g3b = nc.declare_dram_parameter("g3b", [64, N // 2], F32, False)
    out = nc.declare_dram_parameter("out", [B, N // 2], F32, isOutput=True)
    warm_scratch = nc.dram_tensor("warm_scratch", [1, 4], F32)

    with tile.TileContext(nc) as tc:
        with (
            tc.tile_pool(name="main", bufs=1) as pool,
            tc.tile_pool(name="psum", bufs=1, space="PSUM") as pp,
        ):
            # ---- inputs -> SBUF (one serial SP chain; 7 input DMAs) ----
            in_dmas = []
            hdr_sb = pool.tile([128, 128 // 2], F32, tag="hdr", name="hdr")
            in_dmas.append(nc.sync.dma_start(hdr_sb[:], hdr[:, :]))
            kt_sb = [pool.tile([128, CW], F32, tag=f"kt{q}", name=f"kt{q}")
                     for q in range(N_KT_DMA)]
            for q in range(N_KT_DMA):
                in_dmas.append(nc.sync.dma_start(kt_sb[q][:], kts[q][:, :]))
            dbg_sb = pool.tile([128, (KC_CH * 2 * B + N) // 2], F32, tag="dbg", name="dbg")
            in_dmas.append(nc.sync.dma_start(dbg_sb[:], dbg[:, :]))
            g3b_sb = pool.tile([64, N // 2], F32, tag="g3b", name="g3b")
            in_dmas.append(nc.sync.dma_start(g3b_sb[:], g3b[:, :]))

            hdr_v = hdr_sb.bitcast(BF16)          # [128, 128] identity
            kt_v = [t.bitcast(BF16) for t in kt_sb]  # each [128, 2*(1024+128)]
            ktc = [kt_v[c // 2][:, (c % 2) * CW:(c % 2) * CW + N]
                   for c in range(KC_CH)]
            cc_sb = [kt_v[c // 2][:, (c % 2) * CW + N:(c % 2 + 1) * CW]
                     for c in range(KC_CH)]
            dbg_v = dbg_sb.bitcast(BF16)          # [128, 2048 + 1024]
            g3b_v = g3b_sb.bitcast(BF16)          # [64, 1024]

            # DVE staging: operands later consumed alongside DVE-produced
            # tiles must themselves be DVE-produced (one-semaphore rule).
            id_sb = pool.tile([128, 128], BF16, tag="id2", name="id2")
            nc.vector.tensor_copy(id_sb[:], hdr_v[:, 0:128])
            dbt_st = pool.tile([128, KC_CH, 2 * B], BF16, tag="dbtst", name="dbtst")
            nc.vector.tensor_copy(
                dbt_st[:], dbg_v[:, :KC_CH * 2 * B].rearrange(
                    "p (c w) -> p c w", c=KC_CH))
            dbtc = [dbt_st[:, c, :] for c in range(KC_CH)]
            g3a_st = pool.tile([128, N], BF16, tag="g3ast", name="g3ast")
            nc.vector.tensor_copy(g3a_st[:], dbg_v[:, KC_CH * 2 * B:])
            g3b_st = pool.tile([64, N], BF16, tag="g3bst", name="g3bst")
            nc.vector.tensor_copy(g3b_st[:], g3b_v[:])
            g3a_sb = g3a_st
            g3b_v2 = g3b_st

            # ---- PE warmup (keeps the clock at 2.4 GHz while DMAs land) ----
            wz = pool.tile([128, 640], BF16, tag="wz", name="wz")
            nc.gpsimd.memset(wz[:], 0.0)
            wps = pp.tile([128, 512], F32, tag="wps", name="wps")
            for w in range(warm_iters):
                nc.tensor.matmul(wps[:], wz[:, :128], wz[:, 128:640],
                                 start=True, stop=True)
            wsb = pool.tile([128, 4], F32, tag="wsb", name="wsb")
            nc.vector.tensor_copy(wsb[:], wps[:, :4])
            warm_dma = nc.gpsimd.dma_start(warm_scratch[:, :], wsb[:1, :])

            # ---- S1: KC^T[s, k] accumulated over j-chunks ----
            # PSUM is 8 banks x 2KB: big [128, 1024] f32 tile shared by S1
            # and S4 (tag "pskc"), one bf16 bank for all transpose outputs.
            trall = pp.tile([128, 8, 128], BF16, tag="trall", name="trall")
            trall2 = pp.tile([128, 8, 128], BF16, tag="trall2", name="trall2")
            ps_kc = pp.tile([128, D_IN], F32, tag="pskc", name="pskc")
            for c in range(KC_CH):
                for h in range(2):  # matmul out must stay within a PSUM bank
                    nc.tensor.matmul(ps_kc[:, h * 512:(h + 1) * 512],
                                     cc_sb[c], ktc[c][:, h * 512:(h + 1) * 512],
                                     start=(c == 0), stop=(c == KC_CH - 1))
            kcT_sb = pool.tile([128, D_IN], BF16, tag="kcT", name="kcT")
            nc.vector.tensor_copy(kcT_sb[:], ps_kc[:])

            # ---- T1: transpose KC^T chunks -> KC[k, s] chunks ----
            # All 8 into distinct trall slots, then ONE copy (a per-chunk
            # copy after each transpose needs 2 sync waits -> walrus ICE).
            kc_sb = pool.tile([128, KC_CH, S], BF16, tag="kc", name="kc")
            for c in range(KC_CH):
                nc.tensor.transpose(trall[:, c, :],
                                    kcT_sb[:, c * 128:(c + 1) * 128], id_sb)
            nc.vector.tensor_copy(kc_sb[:], trall[:])

            # ---- S2: DB[s, 2B] = KC^T(stationary KC chunks) @ [desT|bodyT] ----
            ps_db = pp.tile([128, 2 * B], F32, tag="psdb", name="psdb")
            for c in range(KC_CH):
                nc.tensor.matmul(ps_db[:], kc_sb[:, c, :], dbtc[c],
                                 start=(c == 0), stop=(c == KC_CH - 1))
            db_sb = pool.tile([128, 2 * B], BF16, tag="db", name="db")
            nc.vector.tensor_copy(db_sb[:], ps_db[:])

            # ---- T2: dT[b, s], bT[b, s] ----
            ps_dt = trall2[:, 0, :]
            ps_bt = trall2[:, 1, :]
            nc.tensor.transpose(ps_dt, db_sb[:, 0:B], id_sb)
            nc.tensor.transpose(ps_bt, db_sb[:, B:2 * B], id_sb)

            # ---- PW: ptA = [Dr*Br | Di*Bi], ptC = Dr*Bi + Di*Br ----
            # (DVE reads at most one PSUM operand: stage dT in SBUF first)
            dt_sb = pool.tile([128, 128], BF16, tag="dtsb", name="dtsb")
            bt_sb = pool.tile([128, 128], BF16, tag="btsb", name="btsb")
            nc.vector.tensor_copy(dt_sb[:], ps_dt)
            nc.vector.tensor_copy(bt_sb[:], ps_bt)
            ptA = pool.tile([128, 128], BF16, tag="ptA", name="ptA")
            ptC = pool.tile([128, 64], BF16, tag="ptC", name="ptC")
            t2 = pool.tile([128, 64], F32, tag="t2", name="t2")
            t3 = pool.tile([128, 64], F32, tag="t3", name="t3")
            nc.vector.tensor_mul(ptA[:, 0:64], dt_sb[:, 0:64], bt_sb[:, 0:64])
            nc.vector.tensor_mul(ptA[:, 64:128], dt_sb[:, 64:128], bt_sb[:, 64:128])
            nc.vector.tensor_mul(t2[:], dt_sb[:, 0:64], bt_sb[:, 64:128])
            nc.vector.tensor_mul(t3[:], dt_sb[:, 64:128], bt_sb[:, 0:64])
            nc.vector.tensor_add(ptC[:], t2[:], t3[:])

            # ---- T3: PA[s=128, b], PC[s2=64, b] ----
            ps_pa = trall2[:, 2, :]
            ps_pc = trall2[0:64, 3, :]
            nc.tensor.transpose(ps_pa, ptA[:], id_sb)
            nc.tensor.transpose(ps_pc, ptC[:], id_sb)
            pa_sb = pool.tile([128, 128], BF16, tag="pa", name="pa")
            pc_sb = pool.tile([64, 128], BF16, tag="pc", name="pc")
            nc.vector.tensor_copy(pa_sb[:], ps_pa)
            nc.vector.tensor_copy(pc_sb[:], ps_pc)

            # ---- S4: out[b, j] = PA^T @ G3a + PC^T @ G3b ----
            ps_out = pp.tile([128, N], F32, tag="psout", name="psout")
            last_mm = None
            for h in range(2):
                nc.tensor.matmul(ps_out[:, h * 512:(h + 1) * 512],
                                 pa_sb[:], g3a_sb[:, h * 512:(h + 1) * 512],
                                 start=True, stop=False)
                last_mm = nc.tensor.matmul(
                    ps_out[:, h * 512:(h + 1) * 512],
                    pc_sb[:], g3b_v2[:, h * 512:(h + 1) * 512],
                    start=False, stop=True)
            out_sb = pool.tile([128, N], BF16, tag="outsb", name="outsb")
            last_cp = nc.vector.tensor_copy(out_sb[:], ps_out[:])
            out_w = out_sb.bitcast(F32)           # [128, 512]
            store_a = nc.sync.dma_start(out[:, :], out_w[:, :])
            store_b = store_a

            # ---- tail: absorb every outstanding tick into SP's clock ----
            prev = None
            for dep in [*in_dmas, warm_dma, store_a, last_mm, last_cp]:
                dr = nc.sync.drain(fusable=False)
                add_dep_helper(dr.ins, dep.ins, sync=True,
                               reason="tail: absorb tick into SP clock")
                if prev is not None:
                    add_dep_helper(dr.ins, prev.ins, sync=False,
                                   reason="tail: keep drain chain ordered")
                prev = dr

    return nc


def _bf16_pack(a):
    """float32 (P, W) -> bf16 packed two-per-word as float32 (P, W//2)."""
    bf = np.ascontiguousarray(np.asarray(a, np.float32).astype(ml_dtypes.bfloat16))
    return bf.view(np.uint8).reshape(bf.shape[0], -1).view(np.float32)


def _partition_pack(a):
    """(n*128, W) -> (128, n*W): row p = concat of chunk rows p."""
    r, w = a.shape
    n = r // 128
    return np.ascontiguousarray(
        a.reshape(n, 128, w).transpose(1, 0, 2).reshape(128, n * w))


def _constants():
    """Per-core CC [N, S], G3a [128, N], G3b [64, N] float32."""
    j = np.arange(N, dtype=np.float64)
    alt = np.cos(np.pi * j)                     # (-1)^j
    ccs, g3as, g3bs = [], [], []
    for c in range(N_CORES):
        f = np.arange(c * FPC, (c + 1) * FPC, dtype=np.float64)
        ang = 2.0 * np.pi * np.outer(j, f) / N             # (j, t)
        cc_re = np.cos(ang)
        cc_im = -np.sin(ang)
        angT = ang.T                                        # (t, j)
        w = 4.0 / N
        A = w * np.cos(angT)                                # m0 rows
        Bm = -w * np.cos(angT)                              # m1 rows
        C = -w * np.sin(angT)                               # mC rows
        if c == 0:
            cc_im[:, 0] = alt                               # f=512 cos column
            A[0, :] = 2.0 / N                               # m0 = D0*B0
            Bm[0, :] = (2.0 / N) * alt                      # m1 = D512*B512
            C[0, :] = 0.0
        cc = np.concatenate([cc_re, cc_im], axis=1)         # (N, 128)
        g3 = np.concatenate([A, Bm, C], axis=0)             # (192, N)
        ccs.append(np.ascontiguousarray(cc, np.float32))
        g3as.append(np.ascontiguousarray(g3[:128], np.float32))
        g3bs.append(np.ascontiguousarray(g3[128:], np.float32))
    return ccs, g3as, g3bs


def kernel(des, body, kernel):
    global LAST_RESULT
    K = np.asarray(kernel, dtype=np.float32)
    des = np.asarray(des, dtype=np.float32)
    body = np.asarray(body, dtype=np.float32)

    kt_bf = _partition_pack(
        K.T.astype(ml_dtypes.bfloat16).astype(np.float32))  # (128, 8*1024) f32
    dbt = np.concatenate([des.T, body.T], axis=1)       # (1024, 256)
    dbt_pk = _partition_pack(_bf16_pack(dbt))           # (128, 8*128) words
    id_pk = _bf16_pack(np.eye(128, dtype=np.float32))   # (128, 64) words

    ccs, g3as, g3bs = _constants()
    ktqs, dbgs, g3bs_pk = [], [], []
    for c in range(N_CORES):
        cc_bf = _partition_pack(ccs[c])                 # (128, 8*128) f32
        # interleave per chunk: [kt_c (1024) | cc_c (128)] then bf16-pack
        ktcc = np.concatenate(
            [kt_bf.reshape(128, KC_CH, N), cc_bf.reshape(128, KC_CH, S)],
            axis=2).reshape(128, KC_CH * (N + S))
        ktcc_pk = _bf16_pack(ktcc)                      # (128, 8*(1024+128)/2)
        w = ktcc_pk.shape[1] // N_KT_DMA
        ktqs.append([np.ascontiguousarray(ktcc_pk[:, q * w:(q + 1) * w])
                     for q in range(N_KT_DMA)])
        dbgs.append(np.ascontiguousarray(
            np.concatenate([dbt_pk, _bf16_pack(g3as[c])], axis=1)))
        g3bs_pk.append(np.ascontiguousarray(_bf16_pack(g3bs[c])))

    if "nc" not in _nc_cache:
        _nc_cache["nc"] = _build_nc()
    nc = _nc_cache["nc"]

    in_maps = [
        {"hdr": np.ascontiguousarray(id_pk),
         **{f"kt{q}": ktqs[c][q] for q in range(N_KT_DMA)},
         "dbg": dbgs[c],
         "g3b": g3bs_pk[c]}
        for c in range(N_CORES)
    ]
    res = run_bass_kernel_spmd(nc, in_maps, list(range(N_CORES)))
    LAST_RESULT = res
    out = np.zeros((B, N), dtype=np.float32)
    for r in res.results:
        w = np.ascontiguousarray(np.asarray(r["out"], np.float32))
        bf = w.view(np.uint8).reshape(B, -1).view(ml_dtypes.bfloat16)
        out += bf.astype(np.float32)
    return out
